# revision 18
# baseline (speedup 1.0000x reference)
"""VQ-VAE EMA codebook update kernel for 8 Trainium2 NeuronCores.

Problem: nn_VectorQuantizerEMA  (B=32, D=256, H=32, W=32, K=2048)
Sharding: data-parallel over B (4 batches / core), codebook replicated,
AllReduce of [embed_sum | counts | (sum(ze^2), sum(-rowmax))] before the EMA
update.

Per-core algorithm (N_loc = 4096 tokens, 32 token tiles of 128):
  - scores s[n,k] = ze @ embed via fp16-split x3 (h@eh + h@el + l@eh), full
    PE rate, max error ~9e-6 (below the 2.6e-5 min top-2 gap -> exact argmin).
  - m = s - |e_k|^2/2 (DVE add of precomputed broadcast), rowmax via max8,
    idx via max_index (first occurrence == jnp argmin tie rule).
  - S = Sign(m - rowmax) in {-1, 0} (ACT, exact on HW).  onehot = S + 1.
  - z_q: indirect-DMA gather of embedT rows, PE-transpose to [d, tok].
  - embed_sum = ze_td^T @ S + colsum(ze) (PE fp16), counts = 1^T @ S + N.
  - commit_loss = BETA/(N*D) * (sum(ze^2) - 2*sum(rowmax)) since
    dist_n = |ze_n|^2 - 2*max_k m[n,k].
  - One AllReduce of a packed [258, 2048] buffer, then EMA update on-device.
"""

import dataclasses

import numpy as np

B, D, H, W, K = 32, 256, 32, 32, 2048
NCORES = 8
B_LOC = B // NCORES            # 4 batches per core
HW = H * W                     # 1024
N_LOC = B_LOC * HW             # 4096 tokens per core
N_TOT = B * HW                 # 32768 tokens globally
P = 128
T_TILES = N_LOC // P           # 32 token tiles
ST_TILES = 16                  # tiles per supertile
N_ST = T_TILES // ST_TILES     # supertiles
TOK_ST = ST_TILES * P          # tokens per supertile

BETA, DECAY, EPS = 0.25, 0.99, 1e-05
OMD = 1.0 - DECAY

_CACHE = {}


def _re_dt(ap, dt):
    """Reinterpret an AP with a different (same-width) dtype."""
    return dataclasses.replace(ap, tensor=dataclasses.replace(ap.tensor, dtype=dt))


def _build():
    import concourse.bacc as bacc
    import concourse.mybir as mybir
    import concourse.tile as tile
    from concourse.bass import IndirectOffsetOnAxis
    from concourse.masks import make_identity

    dt = mybir.dt
    Alu = mybir.AluOpType
    Act = mybir.ActivationFunctionType
    AX = mybir.AxisListType

    f32, f16, u16, u32, i32 = (dt.float32, dt.float16, dt.uint16,
                               dt.uint32, dt.int32)

    nc = bacc.Bacc(
        "TRN2", target_bir_lowering=False, debug=False, num_devices=NCORES
    )

    # ---------------- DRAM I/O ----------------
    ze_d = nc.dram_tensor("z_e_shard", [B_LOC, D, H, W], f32, kind="ExternalInput")
    emb_d = nc.dram_tensor("embed", [D, K], f32, kind="ExternalInput")
    cs_d = nc.dram_tensor("cluster_size", [K], f32, kind="ExternalInput")
    ea_d = nc.dram_tensor("embed_avg", [D, K], f32, kind="ExternalInput")

    zq_d = nc.dram_tensor("z_q_st", [B_LOC, D, H, W], f32, kind="ExternalOutput")
    idx_d = nc.dram_tensor("idx_out", [N_LOC], i32, kind="ExternalOutput")
    loss_d = nc.dram_tensor("commit_loss", [1], f32, kind="ExternalOutput")
    ne_d = nc.dram_tensor("new_embed", [D, K // NCORES], f32, kind="ExternalOutput")
    ncs_d = nc.dram_tensor("new_cluster_size", [K // NCORES], f32,
                           kind="ExternalOutput")
    nea_d = nc.dram_tensor("new_embed_avg", [D, K // NCORES], f32, kind="ExternalOutput")

    ze_ap = ze_d.ap().rearrange("b d h w -> b d (h w)")   # [4, 256, 1024]
    zq_ap = zq_d.ap().rearrange("b d h w -> b d (h w)")

    with tile.TileContext(nc) as tc:
        dram = tc.alloc_tile_pool(name="dram", bufs=1, space="DRAM")
        embT_dram = dram.tile([K, D], f32, tag="embT", name="embT_dram")
        RROW = D + 2
        rs_in = [dram.tile([RROW * NCORES, K // NCORES], f32, tag=f"rs_in{r}",
                           name=f"rs_in{r}") for r in range(2)]
        rs_out = [dram.tile([RROW, K // NCORES], f32, tag=f"rs_out{r}",
                            name=f"rs_out{r}") for r in range(2)]

        const = tc.alloc_tile_pool(name="const", bufs=1)
        ident = const.tile([P, P], f32, tag="ident", name="ident")
        make_identity(nc, ident)
        ident16 = const.tile([P, P], f16, tag="ident16", name="ident16")
        nc.vector.tensor_copy(ident16, ident)
        ones16 = const.tile([P, 1], f16, tag="ones16", name="ones16")
        nc.vector.memset(ones16, 1.0)
        ones_col = const.tile([P, 1], f32, tag="ones_col", name="ones_col")
        nc.vector.memset(ones_col, 1.0)
        ones_row = const.tile([1, P], f32, tag="ones_row", name="ones_row")
        nc.vector.memset(ones_row, 1.0)
        zeros16 = const.tile([P, 8], f16, tag="zeros16", name="zeros16")
        nc.vector.memset(zeros16, 0.0)
        nege2b = const.tile([P, K], f32, tag="nege2b", name="nege2b")
        # fp16 split of embed: eh + el ~= embed to ~2^-22
        eh_sb, el_sb = [], []
        for dc in range(2):
            eh = const.tile([P, K], f16, tag=f"eh{dc}", name=f"eh_sb{dc}")
            el = const.tile([P, K], f16, tag=f"el{dc}", name=f"el_sb{dc}")
            eh_sb.append(eh)
            el_sb.append(el)

        # persistent accumulators etc. (outlive the main-phase pools)
        small = tc.alloc_tile_pool(name="small", bufs=1)
        esum_sb = []
        for dc in range(2):
            t = small.tile([P, K], f32, tag=f"esum_sb{dc}", name=f"esum_sb{dc}")
            esum_sb.append(t)
        counts_sb = small.tile([1, K], f32, tag="counts_sb", name="counts_sb")
        zsum_acc = small.tile([P, 4 * N_ST], f32, tag="zsum_acc", name="zsum_acc")
        zesq_acc = small.tile([P, 4 * N_ST], f32, tag="zesq", name="zesq_acc")
        rowneg = small.tile([P, T_TILES], f32, tag="rowneg", name="rowneg")
        idx_u_all = small.tile([P, T_TILES], u32, tag="idx_u", name="idx_u_all")

        # transpose pool in PSUM (1 bank x 2) used all over
        tp = tc.alloc_tile_pool(name="tp", bufs=2, space="PSUM")

        # ------------- prologue: embed load/split, embedT, -e2/2 -------------
        pro_ps = tc.alloc_tile_pool(name="pro_ps", bufs=1, space="PSUM")
        pro_sb = tc.alloc_tile_pool(name="pro_sb", bufs=2)

        emb_sb = []
        for dc in range(2):
            e = pro_sb.tile([P, K], f32, tag="emb", name=f"emb_sb{dc}")
            nc.sync.dma_start(e, emb_d.ap()[dc * P:(dc + 1) * P, :])
            emb_sb.append(e)
            nc.scalar.activation(eh_sb[dc], e, Act.Copy)     # f32 -> f16 round
            nc.vector.tensor_sub(el_sb[dc], e, eh_sb[dc])    # residual -> f16

        # e2[k] = sum_d embed[d,k]^2 ; ng = -e2/2 ; broadcast to 128 partitions
        e2_ps = pro_ps.tile([1, K], f32, tag="big", name="e2_ps")
        for dc in range(2):
            sq = pro_sb.tile([P, K], f32, tag="sq", name="emb_sq")
            nc.scalar.activation(sq, emb_sb[dc], Act.Square)
            for ks in range(K // 512):
                sl = slice(ks * 512, (ks + 1) * 512)
                nc.tensor.matmul(
                    out=e2_ps[:, sl], lhsT=ones_col[:, :],
                    rhs=sq[:, sl], start=(dc == 0), stop=(dc == 1),
                )
        ng_row = pro_sb.tile([1, K], f32, tag="ng_row", bufs=1, name="ng_row")
        nc.scalar.activation(ng_row, e2_ps, Act.Copy, scale=-0.5)
        ng_ps = pro_ps.tile([P, K], f32, tag="big", name="ng_ps")
        for ks in range(K // 512):
            sl = slice(ks * 512, (ks + 1) * 512)
            nc.tensor.matmul(out=ng_ps[:, sl], lhsT=ones_row[:, :],
                             rhs=ng_row[:, sl], start=True, stop=True)
        nc.scalar.copy(nege2b, ng_ps)
        pro_ps.release()
        pro_sb.release()

        # ------------- main pools (SBUF) -------------
        ze_pool = tc.alloc_tile_pool(name="ze", bufs=2)
        h_pool = tc.alloc_tile_pool(name="h", bufs=2)
        l_pool = tc.alloc_tile_pool(name="l", bufs=2)
        m_pool = tc.alloc_tile_pool(name="m", bufs=3)
        S_pool = tc.alloc_tile_pool(name="S", bufs=ST_TILES)
        zetd_pool = tc.alloc_tile_pool(name="zetd", bufs=ST_TILES)
        zqT_pool = tc.alloc_tile_pool(name="zqT", bufs=6)
        trash_pool = tc.alloc_tile_pool(name="trash", bufs=1)
        mx_pool = tc.alloc_tile_pool(name="mx", bufs=3)
        zq_sb_pool = tc.alloc_tile_pool(name="zq_sb", bufs=3)

        S_tiles = [None] * T_TILES
        zetd_tiles = [None] * T_TILES

        scores = tc.alloc_tile_pool(name="scores", bufs=2, space="PSUM")
        esum_ps_pool = tc.alloc_tile_pool(name="esum_ps", bufs=1, space="PSUM")

        def load_split(st):
            h_t, l_t = [], []
            for dc in range(2):
                h = h_pool.tile([P, TOK_ST], f16, tag="h", name=f"h_{st}_{dc}")
                l = l_pool.tile([P, TOK_ST], f16, tag="l", name=f"l_{st}_{dc}")
                h_t.append(h)
                l_t.append(l)
                for ch in range(TOK_ST // HW):
                    b = st * (TOK_ST // HW) + ch
                    z = ze_pool.tile([P, HW], f32, tag="ze",
                                     name=f"ze_{st}_{dc}_{ch}")
                    nc.sync.dma_start(z, ze_ap[b, dc * P:(dc + 1) * P, :])
                    csl = slice(ch * HW, (ch + 1) * HW)
                    col = 4 * st + 2 * dc + ch
                    nc.scalar.activation(h[:, csl], z, Act.Copy)
                    nc.vector.tensor_sub(l[:, csl], z, h[:, csl])
                    trash2 = trash_pool.tile([P, HW], f16, tag="trash",
                                             name="zs_trash")
                    nc.scalar.activation(trash2, h[:, csl], Act.Copy,
                                         accum_out=zsum_acc[:, col:col + 1])
                    trash = trash_pool.tile([P, HW], f16, tag="trash",
                                            name="sq_trash")
                    nc.scalar.activation(
                        trash, z, Act.Square,
                        accum_out=zesq_acc[:, col:col + 1],
                    )
            return h_t, l_t

        hl0 = load_split(0)

        # build embedT (gather table) from DRAM blocks; only gathers need it
        ebt_pool = tc.alloc_tile_pool(name="ebt", bufs=3)
        for kc in range(K // P):
            tps = tp.tile([P, D], f32, tag="tp", name="embT_ps")
            for dc in range(2):
                eblk = ebt_pool.tile([P, P], f32, tag="eblk", name="eblk")
                nc.sync.dma_start(
                    eblk, emb_d.ap()[dc * P:(dc + 1) * P, kc * P:(kc + 1) * P])
                nc.tensor.transpose(
                    out=tps[:, dc * P:(dc + 1) * P], in_=eblk,
                    identity=ident[:],
                )
            row = ebt_pool.tile([P, D], f32, tag="embT_row", name="embT_row")
            nc.scalar.copy(row, tps)
            nc.sync.dma_start(embT_dram[kc * P:(kc + 1) * P, :], row)
        ebt_pool.release()

        hl_next = hl0
        for st in range(N_ST):
            h_t, l_t = hl_next

            # ---- scores phase ----
            for tl in range(ST_TILES):
                t = st * ST_TILES + tl
                toks = slice(tl * P, (tl + 1) * P)

                # ze_td via PE transpose of h -> fp16 SBUF
                ztd_ps = tp.tile([P, D], f16, tag="tp", name="ztd_ps")
                for dc in range(2):
                    nc.tensor.transpose(
                        out=ztd_ps[:, dc * P:(dc + 1) * P],
                        in_=h_t[dc][:, toks],
                        identity=ident16[:],
                    )
                ze_td = zetd_pool.tile([P, D], f16, tag="zetd", name="ze_td")
                nc.scalar.copy(ze_td, ztd_ps)
                zetd_tiles[t] = ze_td

                # scores matmul (fp16 x3) into PSUM, 2 half-K chunks
                m_t = m_pool.tile([P, K], f32, tag="m", name="m_t")
                for hf in range(2):
                    sc = scores.tile([P, K // 2], f32, tag="sc", name="sc")
                    for ks in range(2):
                        sl_out = slice(ks * 512, (ks + 1) * 512)
                        sl_emb = slice(hf * 1024 + ks * 512,
                                       hf * 1024 + (ks + 1) * 512)
                        for ci, (lt, rt) in enumerate(
                                ((h_t, eh_sb), (h_t, el_sb), (l_t, eh_sb))):
                            for dc in range(2):
                                nc.tensor.matmul(
                                    out=sc[:, sl_out],
                                    lhsT=lt[dc][:, toks],
                                    rhs=rt[dc][:, sl_emb],
                                    start=(ci == 0 and dc == 0),
                                    stop=(ci == 2 and dc == 1),
                                )
                    # m = s - e2/2
                    hsl = slice(hf * 1024, (hf + 1) * 1024)
                    nc.vector.tensor_add(m_t[:, hsl], sc[:], nege2b[:, hsl])

                # rowmax + argmax + S
                m8 = mx_pool.tile([P, 8], f32, tag="m8", name="m8")
                nc.vector.max(out=m8, in_=m_t[:])
                nc.vector.tensor_scalar_mul(rowneg[:, t:t + 1], m8[:, 0:1], -1.0)

                S_t = S_pool.tile([P, K], f16, tag="S", name="S_t")
                nc.scalar.activation(S_t, m_t, Act.Sign,
                                     bias=rowneg[:, t:t + 1], scale=1.0)
                S_tiles[t] = S_t

                # idx = first 0.0 in S (fp16 2x scan)
                idx8 = mx_pool.tile([P, 8], u16, tag="idx8", name="idx8")
                nc.vector.max_index(out=idx8, in_max=zeros16, in_values=S_t[:])
                nc.vector.tensor_copy(idx_u_all[:, t:t + 1], idx8[:, 0:1])

                # gather z_q rows for this tile; transpose + write out
                zqT_t = zqT_pool.tile([P, D], f32, tag="zqT", name="zqT")
                nc.gpsimd.indirect_dma_start(
                    out=zqT_t[:, :], out_offset=None,
                    in_=embT_dram[:, :],
                    in_offset=IndirectOffsetOnAxis(
                        ap=idx_u_all[:, t:t + 1], axis=0),
                )
                zq_ps = tp.tile([P, D], f32, tag="tp", name="zq_ps")
                for dc in range(2):
                    nc.tensor.transpose(
                        out=zq_ps[:, dc * P:(dc + 1) * P],
                        in_=zqT_t[:, dc * P:(dc + 1) * P],
                        identity=ident[:],
                    )
                zq_sb = zq_sb_pool.tile([P, D], f32, tag="zq_sb", name="zq_sb")
                nc.scalar.copy(zq_sb, zq_ps)
                b = t * P // HW
                hw0 = t * P % HW
                for dc in range(2):
                    nc.sync.dma_start(
                        zq_ap[b, dc * P:(dc + 1) * P, hw0:hw0 + P],
                        zq_sb[:, dc * P:(dc + 1) * P],
                    )
            if st + 1 < N_ST:
                hl_next = load_split(st + 1)

            # ---- embed_sum combos (2-bank chunks, persistent pool) ----
            for dc in range(2):
                for kh in range(2):
                    ep = esum_ps_pool.tile([P, K // 2], f32, tag="big2",
                                           name="esum_ps")
                    for tl in range(ST_TILES):
                        t = st * ST_TILES + tl
                        for ks in range(2):
                            sl = slice(ks * 512, (ks + 1) * 512)
                            ssl = slice(kh * 1024 + ks * 512,
                                        kh * 1024 + (ks + 1) * 512)
                            nc.tensor.matmul(
                                out=ep[:, sl],
                                lhsT=zetd_tiles[t][:, dc * P:(dc + 1) * P],
                                rhs=S_tiles[t][:, ssl],
                                start=(tl == 0), stop=(tl == ST_TILES - 1),
                            )
                    esl = slice(kh * 1024, (kh + 1) * 1024)
                    nc.scalar.copy(esum_sb[dc][:, esl], ep)

            for kh in range(2):
                cp = esum_ps_pool.tile([1, K // 2], f32, tag="big2",
                                       name="counts_ps")
                for tl in range(ST_TILES):
                    t = st * ST_TILES + tl
                    for ks in range(2):
                        sl = slice(ks * 512, (ks + 1) * 512)
                        ssl = slice(kh * 1024 + ks * 512,
                                    kh * 1024 + (ks + 1) * 512)
                        nc.tensor.matmul(
                            out=cp[:, sl], lhsT=ones16[:, :],
                            rhs=S_tiles[t][:, ssl],
                            start=(tl == 0), stop=(tl == ST_TILES - 1),
                        )
                csl = slice(kh * 1024, (kh + 1) * 1024)
                nc.scalar.copy(counts_sb[:, csl], cp)

            # ---- pack this supertile's partial sums; ReduceScatter ----
            KS = K // NCORES
            zsum2 = small.tile([P, 2], f32, tag="zsum2", bufs=2, name="zsum2")
            zs_v = zsum_acc.rearrange("p (s d c) -> p s d c", s=N_ST, d=2)
            for dc in range(2):
                nc.vector.reduce_sum(out=zsum2[:, dc:dc + 1],
                                     in_=zs_v[:, st:st + 1, dc, :], axis=AX.XY)
                nc.vector.tensor_scalar(
                    esum_sb[dc], esum_sb[dc], zsum2[:, dc:dc + 1], None,
                    op0=Alu.add,
                )
                for c in range(NCORES):
                    nc.sync.dma_start(
                        rs_in[st][c * RROW + dc * P:c * RROW + (dc + 1) * P, :],
                        esum_sb[dc][:, c * KS:(c + 1) * KS],
                    )
            pk = small.tile([P, 2], f32, tag="pk", bufs=2, name="pk")
            nc.vector.reduce_sum(out=pk[:, 0:1],
                                 in_=zesq_acc[:, 4 * st:4 * (st + 1)], axis=AX.X)
            nc.vector.reduce_sum(
                out=pk[:, 1:2],
                in_=rowneg[:, st * ST_TILES:(st + 1) * ST_TILES], axis=AX.X)
            pk_ps = tp.tile([2, 1], f32, tag="tp", name="pk_ps")
            nc.tensor.matmul(out=pk_ps[:2, :1], lhsT=pk[:, :],
                             rhs=ones_col[:, :], start=True, stop=True)
            pk2 = small.tile([2, 1], f32, tag="pk2", bufs=2, name="pk2")
            nc.scalar.copy(pk2, pk_ps[:2, :1])
            loss_row_st = small.tile([1, K], f32, tag="loss_row", bufs=2,
                                     name="loss_row_st")
            nc.vector.memset(loss_row_st, 0.0)
            nc.sync.dma_start(loss_row_st[:, 0:2], pk2)
            for c in range(NCORES):
                nc.sync.dma_start(
                    rs_in[st][c * RROW + D:c * RROW + D + 1, :],
                    counts_sb[:, c * KS:(c + 1) * KS],
                )
                nc.sync.dma_start(
                    rs_in[st][c * RROW + D + 1:c * RROW + D + 2, :],
                    loss_row_st[:, c * KS:(c + 1) * KS],
                )
            nc.gpsimd.collective_compute(
                "ReduceScatter", Alu.add,
                replica_groups=[list(range(NCORES))],
                ins=[rs_in[st].opt()],
                outs=[rs_out[st].opt()],
            )

        esum_ps_pool.release()
        scores.release()

        # release the big main-phase SBUF pools (reverse alloc order)
        zq_sb_pool.release()
        mx_pool.release()
        trash_pool.release()
        zqT_pool.release()
        zetd_pool.release()
        S_pool.release()
        m_pool.release()
        l_pool.release()
        h_pool.release()
        ze_pool.release()

        # ---------------- idx output ----------------
        epi0 = tc.alloc_tile_pool(name="epi0", bufs=1)
        idx_f = epi0.tile([P, T_TILES], f32, tag="idx_f", name="idx_f")
        nc.vector.tensor_copy(idx_f, idx_u_all)
        idxT_ps = tp.tile([P, P], f32, tag="tp", name="idxT_ps")
        nc.tensor.transpose(out=idxT_ps[:T_TILES, :], in_=idx_f[:, :],
                            identity=ident[:])
        idxT_i = epi0.tile([T_TILES, P], i32, tag="idxT_i", name="idxT_i")
        nc.vector.tensor_copy(idxT_i, idxT_ps[:T_TILES, :])
        nc.sync.dma_start(
            idx_d.ap().rearrange("(t p) -> t p", t=T_TILES), idxT_i
        )

        # ---------------- epilogue ----------------
        ep_ps = tc.alloc_tile_pool(name="ep_ps", bufs=1, space="PSUM")
        KS = K // NCORES

        # prefetch + pre-scale this core's embed_avg k-slice
        from concourse.bass import ds as dslice
        pid = nc.sync.partition_id()
        tA_pre = []
        for dc in range(2):
            ea_sb = epi0.tile([P, KS], f32, tag=f"ea{dc}", name=f"ea_sb{dc}")
            nc.sync.dma_start(
                ea_sb, ea_d.ap()[dc * P:(dc + 1) * P, dslice(pid * KS, KS)])
            tA = epi0.tile([P, KS], f32, tag=f"tA{dc}", name=f"tA{dc}")
            nc.scalar.activation(tA, ea_sb, Act.Copy, scale=DECAY)
            tA_pre.append(tA)

        # ---------------- EMA update (identical on all cores) ----------------
        epi = tc.alloc_tile_pool(name="epi", bufs=1)
        ar_esum = []
        for dc in range(2):
            a = epi.tile([P, KS], f32, tag=f"ar_esum{dc}", name=f"ar_esum{dc}")
            nc.sync.dma_start(a, rs_out[0][dc * P:(dc + 1) * P, :])
            a2 = epi.tile([P, KS], f32, tag=f"ar_esum{dc}b", name=f"ar_esum{dc}b")
            nc.sync.dma_start(a2, rs_out[1][dc * P:(dc + 1) * P, :])
            nc.vector.tensor_add(a, a, a2)
            ar_esum.append(a)
        lp_a = epi.tile([1, 2], f32, tag="lp_a", name="lp_a")
        nc.sync.dma_start(lp_a, rs_out[0][D + 1:D + 2, 0:2])
        lp_b = epi.tile([1, 2], f32, tag="lp_b", name="lp_b")
        nc.sync.dma_start(lp_b, rs_out[1][D + 1:D + 2, 0:2])
        lp = epi.tile([1, 2], f32, tag="lp", name="lp")
        nc.vector.tensor_add(lp, lp_a, lp_b)

        # counts slice [2, 128] (+N_TOT correction)
        cnt_a = epi.tile([2, P], f32, tag="cnt_a", name="cnt_a")
        nc.sync.dma_start(
            cnt_a, rs_out[0][D:D + 1, :].rearrange("a (p f) -> (a p) f", p=2))
        cnt_b = epi.tile([2, P], f32, tag="cnt_b", name="cnt_b")
        nc.sync.dma_start(
            cnt_b, rs_out[1][D:D + 1, :].rearrange("a (p f) -> (a p) f", p=2))
        counts_sl = epi.tile([2, P], f32, tag="counts_sl", name="counts_sl")
        nc.vector.tensor_add(counts_sl, cnt_a, cnt_b)
        nc.vector.tensor_scalar_add(counts_sl, counts_sl, float(N_TOT))

        # new_cluster_size slice = cs_sl*DECAY + OMD*counts_sl
        cs_sl = epi.tile([2, P], f32, tag="cs_sl", name="cs_sl")
        nc.sync.dma_start(
            cs_sl,
            cs_d.ap().rearrange("(a b) -> a b", b=P)[dslice(pid * 2, 2), :])
        ncs_sl = epi.tile([2, P], f32, tag="ncs_sl", name="ncs_sl")
        tsa = epi.tile([2, P], f32, tag="tsa", name="tsa")
        nc.scalar.activation(tsa, cs_sl, Act.Copy, scale=DECAY)
        tsb = epi.tile([2, P], f32, tag="tsb", name="tsb")
        nc.vector.tensor_scalar_mul(tsb, counts_sl, OMD)
        nc.vector.tensor_add(ncs_sl, tsa, tsb)
        nc.sync.dma_start(ncs_d.ap().rearrange("(a b) -> a b", b=P), ncs_sl)

        # n = DECAY*sum(cs) + OMD*N_TOT  (sum over k of ncs, no collective)
        cs16 = epi.tile([16, P], f32, tag="cs16", name="cs16")
        nc.sync.dma_start(cs16, cs_d.ap().rearrange("(p f) -> p f", p=16))
        nred = epi.tile([16, 1], f32, tag="nred", name="nred")
        nc.vector.reduce_sum(out=nred, in_=cs16, axis=AX.X)
        n_ps = ep_ps.tile([1, 1], f32, tag="tiny", name="n_ps")
        nc.tensor.matmul(out=n_ps[:, :], lhsT=nred[:, :],
                         rhs=ones_col[0:16, :], start=True, stop=True)
        n_sc = epi.tile([1, 1], f32, tag="n_sc", name="n_sc")
        nc.scalar.copy(n_sc, n_ps)
        nc.vector.tensor_scalar(n_sc, n_sc, DECAY, float(OMD * N_TOT),
                                op0=Alu.mult, op1=Alu.add)
        n2_ps = ep_ps.tile([2, 1], f32, tag="tiny", name="n2_ps")
        nc.tensor.matmul(out=n2_ps[:, :], lhsT=ones_row[:, 0:2],
                         rhs=n_sc[:, :], start=True, stop=True)
        n2 = epi.tile([2, 1], f32, tag="n2", name="n2")
        nc.scalar.copy(n2, n2_ps)
        den2 = epi.tile([2, 1], f32, tag="den2", name="den2")
        nc.vector.tensor_scalar_add(den2, n2, float(K * EPS))
        rec2 = epi.tile([2, 1], f32, tag="rec2", name="rec2")
        nc.vector.reciprocal(rec2, den2)
        r2 = epi.tile([2, 1], f32, tag="r2", name="r2")
        nc.vector.tensor_mul(r2, n2, rec2)
        csn_sl = epi.tile([2, P], f32, tag="csn_sl", name="csn_sl")
        nc.vector.tensor_scalar(csn_sl, ncs_sl, float(EPS), r2[:, :],
                                op0=Alu.add, op1=Alu.mult)
        inv_sl = epi.tile([2, P], f32, tag="inv_sl", name="inv_sl")
        nc.vector.reciprocal(inv_sl, csn_sl)
        inv_row = epi.tile([1, KS], f32, tag="inv_row", name="inv_row")
        nc.sync.dma_start(inv_row, inv_sl)

        # broadcast 1/cs to 128 partitions (C=1 fp32 matmul)
        invb_ps = ep_ps.tile([P, KS], f32, tag="invb", name="invb_ps")
        nc.tensor.matmul(out=invb_ps[:, :], lhsT=ones_row[:, :],
                         rhs=inv_row[:, :], start=True, stop=True)

        # new_embed_avg = ea*DECAY + OMD*esum ; new_embed = nea * (1/cs)
        for dc in range(2):
            tA = tA_pre[dc]
            tB = epi.tile([P, KS], f32, tag="tB", name=f"tB{dc}")
            nc.vector.tensor_scalar_mul(tB, ar_esum[dc], OMD)
            nea_sb = epi.tile([P, KS], f32, tag="nea", name=f"nea_sb{dc}")
            nc.vector.tensor_add(nea_sb, tA, tB)
            nc.sync.dma_start(nea_d.ap()[dc * P:(dc + 1) * P, :], nea_sb)
            ne_sb = epi.tile([P, KS], f32, tag="ne", name=f"ne_sb{dc}")
            nc.vector.tensor_mul(ne_sb, nea_sb, invb_ps)
            nc.sync.dma_start(ne_d.ap()[dc * P:(dc + 1) * P, :], ne_sb)

        # ---------------- commit loss ----------------
        # loss = BETA/(N*D) * (sum(ze^2) + 2*sum(-rowmax))
        lossv = epi.tile([1, 1], f32, tag="lossv", name="lossv")
        nc.vector.tensor_scalar(lossv, lp[:, 1:2], 2.0, None, op0=Alu.mult)
        nc.vector.tensor_add(lossv, lossv, lp[:, 0:1])
        nc.vector.tensor_scalar_mul(lossv, lossv, float(BETA / (N_TOT * D)))
        nc.sync.dma_start(loss_d.ap().rearrange("(a b) -> a b", a=1), lossv)

        ep_ps.release()
        epi.release()
        epi0.release()
        tp.release()
        small.release()
        const.release()
        dram.release()

    nc.compile()
    return nc


def _get_nc():
    if "nc" not in _CACHE:
        _CACHE["nc"] = _build()
    return _CACHE["nc"]


def kernel(z_e, embed, cluster_size, embed_avg, trace=False):
    import concourse.bass_utils as bass_utils

    z_e = np.ascontiguousarray(np.asarray(z_e, dtype=np.float32))
    embed = np.ascontiguousarray(np.asarray(embed, dtype=np.float32))
    cluster_size = np.ascontiguousarray(np.asarray(cluster_size, dtype=np.float32))
    embed_avg = np.ascontiguousarray(np.asarray(embed_avg, dtype=np.float32))

    nc = _get_nc()
    in_maps = []
    for c in range(NCORES):
        in_maps.append({
            "z_e_shard": np.ascontiguousarray(z_e[c * B_LOC:(c + 1) * B_LOC]),
            "embed": embed,
            "cluster_size": cluster_size,
            "embed_avg": embed_avg,
        })
    res = bass_utils.run_bass_kernel_spmd(
        nc, in_maps, core_ids=list(range(NCORES)), trace=trace
    )
    _CACHE["last_result"] = res
    outs = res.results

    z_q_st = np.concatenate([outs[c]["z_q_st"] for c in range(NCORES)], axis=0)
    idx = np.concatenate(
        [outs[c]["idx_out"].reshape(B_LOC, H, W) for c in range(NCORES)], axis=0
    ).astype(np.int32)
    commit_loss = np.float32(outs[0]["commit_loss"][0])
    new_embed = np.concatenate([outs[c]["new_embed"] for c in range(NCORES)],
                               axis=1)
    new_cluster_size = np.concatenate(
        [outs[c]["new_cluster_size"] for c in range(NCORES)], axis=0)
    new_embed_avg = np.concatenate(
        [outs[c]["new_embed_avg"] for c in range(NCORES)], axis=1)
    return (z_q_st, idx, commit_loss, new_embed, new_cluster_size, new_embed_avg)
